# revision 25
# baseline (speedup 1.0000x reference)
"""Trainium2 Bass kernel for nn_Attention_19404662243470.

Sharding: 8 cores = (batch 2) x (heads 4). Each core computes the full
attention pipeline for its (b, h) pair in transposed layout [d, n]; the
final pointwise conv partials are ReduceScattered (8 chunked pieces)
within each batch's 4-core group, and LayerNorm2d runs per chunk on the
scattered shard, interleaved with the attention loop.

Restructure (494us baseline -> ~338us):
 - x loaded with contiguous DMAs split over 4 queues; the padded conv
   grid is built on-chip fused with the positional-encoding add (the
   old strided-x DMA was 256B-line bound and stalled the first 40us);
   the grid guard ring is zeroed with tiny DVE copies (a DMA'd guard
   ring cost ~3us of descriptor generation per edge).
 - q/k LayerNorm runs per 512-column chunk entirely in [128, 512]
   tiles: mean-centering is FOLDED INTO THE POINTWISE WEIGHTS (the
   column mean of q is a linear function of the dw-conv output, so
   pwq - colmean(pwq) yields centered q directly); variance =
   mean((x-mu)^2) via one gpsimd square + one ones-matmul broadcast.
   No [1, N] single-partition chains (the old ones serialized ~40us
   on 1 of 128 lanes), no post-LN mirror copies (pointwise weights are
   pre-duplicated so every tile is born [128, *] with both PE halves).
 - softmax denominator is never divided out: out = attnV/den + v feeds
   a LayerNorm, which is scale-invariant, so we normalize
   den*out = attnV + den*v instead (saves the reciprocal chain).
 - 3 of 16 exp groups per chunk are offloaded from ACT to the DVE with
   a Schraudolph bitcast exp (fp32 affine -> int16 -> bf16 bits, ~3%
   per-weight error that averages out in the AV sum) to keep the ACT
   exp stream off the critical path.
 - ReduceScatter in 4 pieces issued inside the loop; all LayerNorm2d +
   output DMA after the loop (an RS consumes its whole DRAM tensor, so
   anything queued behind it that touches rs_in/rs_out stalls on peer
   skew - keep such ops off the mid-loop queues).
"""

import numpy as np

import concourse.bass as bass
import concourse.tile as tile
from concourse import bacc, mybir
from concourse.bass_utils import run_bass_kernel_spmd

dt = mybir.dt
AF = mybir.ActivationFunctionType
OP = mybir.AluOpType

B, DIM, Hs, Ws = 2, 128, 64, 64
HEADS, DH = 4, 64
N = Hs * Ws  # 4096
EPS = 1e-6
IC = 512  # i-chunk width
NIC = N // IC  # 8
JB = 128  # j-block
NJB = N // JB  # 32
NCH = N // 128  # 32
G = Hs + 2  # 66 padded grid

# Schraudolph exp on DVE: exp(s/8) ~= bitcast_bf16(int16(s*SEXP_A + SEXP_B))
SEXP_A = 1.4426950408889634 / 8.0 * (1 << 7)
SEXP_B = 127.0 * (1 << 7) - 365000.0 / 65536.0
# j-group pairs (of 16) whose exp runs on DVE instead of ACT
OFFLOAD_G = (3, 7, 11, 14)

_TABLES_PATCHED = False


def _patch_act_tables():
    """Restrict Exp/Ln to the natural_log_exp_and_others set so the ACT
    table never reloads between the softmax Exp stream and the LN-chain
    Ln/Exp pairs (a reload costs ~2.7us and stalls the exp feed)."""
    global _TABLES_PATCHED
    if _TABLES_PATCHED:
        return
    from concourse import bacc as _bacc_mod

    orig = _bacc_mod.get_activation_tables

    def patched(arch):
        tabs = dict(orig(arch))
        keep = {mybir.ActivationFunctionType.Exp, mybir.ActivationFunctionType.Ln}
        return {
            name: (fns if name == "natural_log_exp_and_others" else fns - keep)
            for name, fns in tabs.items()
        }

    _bacc_mod.get_activation_tables = patched
    _TABLES_PATCHED = True


def _build(flags):
    # flags: (qk_w_fold, qk_general, o_general) booleans
    qk_w_fold, qk_general, o_general = flags
    _patch_act_tables()
    nc = bacc.Bacc()

    def par(name, shape, dtyp=dt.float32):
        return nc.declare_dram_parameter(name, list(shape), dtyp, isOutput=False)

    x = par("x", [DIM, N])
    meshb = par("meshb", [3, N], dt.bfloat16)
    pewT = par("pewT", [3, DIM], dt.bfloat16)
    qdiags = par("qdiags", [DIM, 9 * DIM], dt.float32r)
    pwq2 = par("pwq2", [DIM, 128], dt.float32r)
    pwk2 = par("pwk2", [DIM, 128], dt.float32r)
    pwv = par("pwv", [DIM, DH], dt.float32r)
    ones_var = par("ones_var", [DH, 128], dt.float32r)  # 1/64 everywhere
    onesr1 = par("onesr1", [1, DH], dt.float32r)  # ones row
    odiags = par("odiags", [DH, 9 * DH], dt.bfloat16)
    opw = par("opw", [DH, DIM], dt.bfloat16)
    ln2w = par("ln2w", [1, DIM])
    ln2b = par("ln2b", [1, DIM])
    zpad = par("zpad", [DIM, G], dt.float32r)
    if qk_w_fold or qk_general:
        wqk = par("wqk", [128, 1])
        bq_t = par("bq_t", [128, 1])
        wk_t = par("wk_t", [128, 1])
        bk_t = par("bk_t", [128, 1])
    if o_general:
        wo_t = par("wo_t", [DH, 1])
        bo_t = par("bo_t", [DH, 1])
    out_ext = nc.declare_dram_parameter("out", [N // 4, DIM], dt.float32, isOutput=True)

    rs_in = nc.dram_tensor("rs_in", [N, DIM], dt.bfloat16)
    rs_out = nc.dram_tensor("rs_out", [N // 4, DIM], dt.bfloat16)

    with (
        nc.allow_low_precision(reason="float32r/bf16 compute by design"),
        tile.TileContext(nc) as tc,
        tc.tile_pool(name="main", bufs=1) as main,
        tc.tile_pool(name="tmp2", bufs=2) as tmp2,
    ):
        # ---- persistent SBUF tiles ----
        QL = main.tile([128, N], dt.float32r)  # LN'd q, both PE halves
        KL = main.tile([128, N], dt.float32r)
        VT = main.tile([DH, N], dt.float32r)  # v in [d, n] for the skip
        V = main.tile([128, NCH, DH + 1], dt.bfloat16)
        Yr = main.tile([DIM, N], dt.float32r)
        XF = main.tile([DIM, N], dt.float32)
        Xg = main.tile([DIM, G, G], dt.float32r)
        Og = main.tile([DH, G, G], dt.bfloat16)
        DWO = main.tile([DH, N], dt.bfloat16)
        odg = main.tile([DH, 9, DH], dt.bfloat16)
        opw_t = main.tile([DH, DIM], dt.bfloat16)
        mesh_t = main.tile([3, N], dt.bfloat16)
        pwt = main.tile([3, DIM], dt.bfloat16)
        pwq_t = main.tile([DIM, 128], dt.float32r)
        pwk_t = main.tile([DIM, 128], dt.float32r)
        pwv_t = main.tile([DIM, DH], dt.float32r)
        onesv = main.tile([DH, 128], dt.float32r)
        ones1 = main.tile([1, DH], dt.float32r)
        epsP = main.tile([128, 1], dt.float32)

        for _i, _eng in enumerate((nc.scalar, nc.sync, nc.gpsimd, nc.scalar)):
            _eng.dma_start(
                out=XF[:, _i * 1024 : (_i + 1) * 1024],
                in_=x[:, _i * 1024 : (_i + 1) * 1024],
            )
        nc.gpsimd.dma_start(out=mesh_t, in_=meshb[:, :])
        nc.gpsimd.dma_start(out=pwt, in_=pewT[:, :])
        qdg = main.tile([DIM, 9, DIM], dt.float32r)
        nc.sync.dma_start(
            out=qdg, in_=qdiags[:, :].rearrange("p (t c) -> p t c", t=9)
        )
        nc.sync.dma_start(out=pwq_t, in_=pwq2[:, :])
        nc.sync.dma_start(out=pwk_t, in_=pwk2[:, :])
        nc.sync.dma_start(out=pwv_t, in_=pwv[:, :])
        nc.sync.dma_start(out=onesv, in_=ones_var[:, :])
        nc.sync.dma_start(out=ones1, in_=onesr1[:, :])
        nc.scalar.dma_start(
            out=odg, in_=odiags[:, :].rearrange("p (t c) -> p t c", t=9)
        )
        nc.scalar.dma_start(out=opw_t, in_=opw[:, :])
        if qk_w_fold or qk_general:
            wqk_t = main.tile([128, 1], dt.float32)
            bqv = main.tile([128, 1], dt.float32)
            wkv = main.tile([128, 1], dt.float32)
            bkv = main.tile([128, 1], dt.float32)
            nc.sync.dma_start(out=wqk_t, in_=wqk[:, :])
            nc.sync.dma_start(out=bqv, in_=bq_t[:, :])
            nc.sync.dma_start(out=wkv, in_=wk_t[:, :])
            nc.sync.dma_start(out=bkv, in_=bk_t[:, :])
        if o_general:
            wov = main.tile([DH, 1], dt.float32)
            bov = main.tile([DH, 1], dt.float32)
            nc.sync.dma_start(out=wov, in_=wo_t[:, :])
            nc.sync.dma_start(out=bov, in_=bo_t[:, :])
        nc.vector.memset(epsP, EPS)
        nc.vector.memset(V, 1.0)
        nc.vector.memset(Og, 0.0)
        Zr = main.tile([DIM, G], dt.float32)
        nc.vector.memset(Zr, 0.0)
        nc.vector.tensor_copy(out=Xg[:, 0:1, :], in_=Zr[:, :].unsqueeze(1))
        nc.vector.tensor_copy(out=Xg[:, G - 1 : G, :], in_=Zr[:, :].unsqueeze(1))
        nc.vector.tensor_copy(
            out=Xg[:, 1 : G - 1, 0:1], in_=Zr[:, 0 : G - 2].unsqueeze(2)
        )
        nc.vector.tensor_copy(
            out=Xg[:, 1 : G - 1, G - 1 : G], in_=Zr[:, 0 : G - 2].unsqueeze(2)
        )

        # ============ Stage A: grid + pos, depthwise, q/k LN, v ============
        with tc.tile_pool(name="psA1", bufs=2, space="PSUM") as psA1:
            # pos encoding fused with grid fill
            for c in range(NIC):
                pos_ps = psA1.tile([DIM, IC], dt.float32, tag="posdw")
                nc.tensor.matmul(
                    pos_ps, pwt, mesh_t[:, c * IC : (c + 1) * IC], start=True, stop=True
                )
                r0 = c * 8
                nc.vector.tensor_add(
                    out=Xg[:, 1 + r0 : 9 + r0, 1 : 1 + Ws],
                    in0=XF[:, c * IC : (c + 1) * IC].rearrange(
                        "p (a b) -> p a b", b=Ws
                    ),
                    in1=pos_ps.rearrange("p (a b) -> p a b", b=Ws),
                )
            # qkv depthwise 3x3 via 9 accumulated diag matmuls
            for c in range(NIC):
                dwp = psA1.tile([DIM, IC], dt.float32, tag="posdw")
                r0 = c * 8
                t = 0
                for di in range(3):
                    for dj in range(3):
                        nc.tensor.matmul(
                            dwp,
                            qdg[:, t, :],
                            Xg[:, r0 + di : r0 + di + 8, dj : dj + Ws],
                            start=(t == 0),
                            stop=(t == 8),
                        )
                        t += 1
                nc.scalar.copy(out=Yr[:, c * IC : (c + 1) * IC], in_=dwp)

        with (
            tc.tile_pool(name="psA", bufs=1, space="PSUM") as psA,
            tc.tile_pool(name="sbA", bufs=4) as sbA,
        ):

            def ln_side(c, pw_w, dst, is_q):
                """centered pointwise + per-chunk LN into dst[:, c*IC:+IC].

                pw_w is pre-centered on host (columns minus their mean), so
                the matmul directly yields D = x - mean(x) per column."""
                sl = slice(c * IC, (c + 1) * IC)
                yc = Yr[:, sl]
                pp = psA.tile([128, IC], dt.float32, tag="pw", bufs=4)
                nc.tensor.matmul(pp, pw_w, yc, start=True, stop=True)
                D = sbA.tile([128, IC], dt.float32r, tag="D")
                nc.vector.tensor_copy(out=D, in_=pp)
                SQ = sbA.tile([DH, IC], dt.float32r, tag="SQ")
                nc.gpsimd.tensor_mul(out=SQ, in0=D[0:DH, :], in1=D[0:DH, :])
                varb = psA.tile([128, IC], dt.float32, tag="varb", bufs=2)
                nc.tensor.matmul(varb, onesv, SQ, start=True, stop=True)
                LNV = sbA.tile([128, IC], dt.float32, tag="LNV")
                nc.scalar.activation(out=LNV, in_=varb, func=AF.Ln, bias=epsP)
                RS = sbA.tile([128, IC], dt.float32, tag="RS")
                nc.scalar.activation(out=RS, in_=LNV, func=AF.Exp, scale=-0.5)
                nc.vector.tensor_mul(out=dst[:, sl], in0=D, in1=RS)
                if qk_general:
                    w_ap, b_ap = (wqk_t, bqv) if is_q else (wkv, bkv)
                    nc.vector.tensor_scalar(
                        out=dst[:, sl], in0=dst[:, sl],
                        scalar1=w_ap, scalar2=b_ap, op0=OP.mult, op1=OP.add,
                    )
                elif qk_w_fold and is_q:
                    nc.vector.tensor_scalar_mul(
                        out=dst[:, sl], in0=dst[:, sl], scalar1=wqk_t
                    )

            for c in range(NIC):
                ln_side(c, pwk_t, KL, False)
                ln_side(c, pwq_t, QL, True)
                # v: [d, n] for the skip and [token, d] for AV stationary
                sl = slice(c * IC, (c + 1) * IC)
                vtp = psA.tile([DH, IC], dt.float32, tag="vtp", bufs=1)
                nc.tensor.matmul(vtp, pwv_t, Yr[:, sl], start=True, stop=True)
                nc.scalar.copy(out=VT[:, sl], in_=vtp)
                vp = psA.tile([128, 4, DH], dt.float32, tag="vp", bufs=1)
                for j in range(4):
                    ch = 4 * c + j
                    nc.tensor.matmul(
                        vp[:, j, :],
                        Yr[:, ch * 128 : (ch + 1) * 128],
                        pwv_t,
                        start=True,
                        stop=True,
                    )
                nc.vector.tensor_copy(out=V[:, 4 * c : 4 * c + 4, 0:DH], in_=vp)

        # ============ Stage B: attention + inline out-LN + out-conv ============
        with tc.tile_pool(name="psB", bufs=1, space="PSUM") as psB, tc.tile_pool(
            name="sbB", bufs=3
        ) as sbB:
            NG = NJB // 2
            w_b = main.tile([128, DIM], dt.float32)
            b_b = main.tile([128, DIM], dt.float32)
            nc.sync.dma_start(out=w_b, in_=ln2w[:, :].to_broadcast([128, DIM]))
            nc.sync.dma_start(out=b_b, in_=ln2b[:, :].to_broadcast([128, DIM]))

            def attention_block(c, pre_cb=None):
                avp = psB.tile([DH + 1, IC], dt.float32, tag="avp", bufs=1)
                stgs = {}
                Es = {}

                def issue_st(g):
                    stg = psB.tile([128, 2 * IC], dt.float32, tag="stg", bufs=2)
                    j0 = 2 * g * JB
                    nc.tensor.matmul(
                        stg[:, 0:IC],
                        KL[0:DH, j0 : j0 + JB],
                        QL[0:DH, c * IC : (c + 1) * IC],
                        start=True,
                        stop=True,
                    )
                    nc.tensor.matmul(
                        stg[:, IC : 2 * IC],
                        KL[DH:128, j0 + JB : j0 + 2 * JB],
                        QL[DH:128, c * IC : (c + 1) * IC],
                        start=True,
                        stop=True,
                    )
                    stgs[g] = stg

                def issue_exp(g):
                    if g in OFFLOAD_G:
                        EI = sbB.tile([128, 2 * IC], dt.int16, tag="EI", bufs=3)
                        nc.vector.tensor_scalar(
                            out=EI,
                            in0=stgs.pop(g),
                            scalar1=SEXP_A,
                            scalar2=SEXP_B,
                            op0=OP.mult,
                            op1=OP.add,
                        )
                        Es[g] = EI.bitcast(dt.bfloat16)
                    else:
                        E = sbB.tile([128, 2 * IC], dt.bfloat16, tag="E", bufs=4)
                        nc.scalar.activation(
                            out=E, in_=stgs.pop(g), func=AF.Exp, scale=float(DH**-0.5)
                        )
                        Es[g] = E

                def issue_av(g):
                    E = Es.pop(g)
                    for t in range(2):
                        jb = 2 * g + t
                        nc.tensor.matmul(
                            avp,
                            V[:, jb, :],
                            E[:, t * IC : (t + 1) * IC],
                            start=(jb == 0),
                            stop=(jb == NJB - 1),
                            skip_group_check=True,
                        )

                issue_st(0)
                issue_exp(0)
                if pre_cb is not None:
                    pre_cb()
                for g in range(1, NG):
                    issue_st(g)
                    issue_exp(g)
                    issue_av(g - 1)
                issue_av(NG - 1)
                return avp

            def drain_avp(c, avp):
                # consume avp (bufs=1): build den*v + attnV for the out-LN
                sl = slice(c * IC, (c + 1) * IC)
                DEN = sbB.tile([1, IC], dt.float32r, tag="DEN", bufs=2)
                nc.vector.tensor_copy(out=DEN, in_=avp[DH : DH + 1, :])
                denb = psB.tile([DH, IC], dt.float32, tag="ob", bufs=1)
                nc.tensor.matmul(denb, ones1, DEN, start=True, stop=True)
                OS = sbB.tile([DH, IC], dt.float32r, tag="OS", bufs=2)
                nc.vector.tensor_mul(out=OS, in0=VT[:, sl], in1=denb)
                nc.vector.tensor_add(out=OS, in0=OS, in1=avp[0:DH, :])
                return OS

            def tail_block(c, OS):
                """out-LN chain for chunk c (scale-invariant, no division)."""
                mub = psB.tile([DH, IC], dt.float32, tag="ob", bufs=1)
                nc.tensor.matmul(mub, onesv[:, 0:DH], OS, start=True, stop=True)
                Do = sbB.tile([DH, IC], dt.float32r, tag="Do", bufs=2)
                nc.vector.scalar_tensor_tensor(
                    out=Do, in0=OS, scalar=1.0, in1=mub, op0=OP.mult, op1=OP.subtract
                )
                SQo = sbB.tile([DH, IC], dt.float32r, tag="SQo", bufs=2)
                nc.vector.tensor_mul(out=SQo, in0=Do, in1=Do)
                varb = psB.tile([DH, IC], dt.float32, tag="ob", bufs=1)
                nc.tensor.matmul(varb, onesv[:, 0:DH], SQo, start=True, stop=True)
                LNV = sbB.tile([DH, IC], dt.float32, tag="LNVo", bufs=2)
                nc.scalar.activation(out=LNV, in_=varb, func=AF.Ln, bias=epsP[0:DH, :])
                RSo = sbB.tile([DH, IC], dt.float32, tag="RSo", bufs=2)
                nc.scalar.activation(out=RSo, in_=LNV, func=AF.Exp, scale=-0.5)
                r0 = c * 8
                gview = Og[:, 1 + r0 : 9 + r0, 1 : 1 + Ws]
                if o_general:
                    T = sbB.tile([DH, IC], dt.float32, tag="To", bufs=2)
                    nc.vector.tensor_mul(out=T, in0=Do, in1=RSo)
                    nc.vector.tensor_scalar(
                        out=gview,
                        in0=T.rearrange("p (a b) -> p a b", b=Ws),
                        scalar1=wov,
                        scalar2=bov,
                        op0=OP.mult,
                        op1=OP.add,
                    )
                else:
                    nc.vector.tensor_mul(
                        out=gview,
                        in0=Do.rearrange("p (a b) -> p a b", b=Ws),
                        in1=RSo.rearrange("p (a b) -> p a b", b=Ws),
                    )

            def dw_chunk(c):
                dwp = psB.tile([DH, IC], dt.float32, tag="dw", bufs=1)
                r0 = c * 8
                t = 0
                for di in range(3):
                    for dj in range(3):
                        nc.tensor.matmul(
                            dwp,
                            odg[:, t, :],
                            Og[:, r0 + di : r0 + di + 8, dj : dj + Ws],
                            start=(t == 0),
                            stop=(t == 8),
                        )
                        t += 1
                nc.vector.tensor_copy(out=DWO[:, c * IC : (c + 1) * IC], in_=dwp)
                pp = psB.tile([128, 4, DIM], dt.float32, tag="pp", bufs=1)
                for j in range(4):
                    ch = 4 * c + j
                    nc.tensor.matmul(
                        pp[:, j, :],
                        DWO[:, ch * 128 : (ch + 1) * 128],
                        opw_t,
                        start=True,
                        stop=True,
                    )
                PP = tmp2.tile([128, 4, DIM], dt.bfloat16, tag="PP")
                nc.vector.tensor_copy(out=PP, in_=pp)
                nc.gpsimd.dma_start(
                    out=rs_in[c * IC : (c + 1) * IC, :].rearrange(
                        "(t p) d -> p t d", p=128
                    ),
                    in_=PP,
                )
                if c % 2 == 1:
                    p = c // 2
                    nc.gpsimd.collective_compute(
                        "ReduceScatter",
                        OP.add,
                        replica_groups=[[0, 1, 2, 3], [4, 5, 6, 7]],
                        ins=[rs_in[p * 1024 : (p + 1) * 1024, :]],
                        outs=[rs_out[p * 256 : (p + 1) * 256, :]],
                    )

            def stageD_chunk(c):
                R = tmp2.tile([128, DIM], dt.bfloat16, tag="Rb")
                nc.sync.dma_start(out=R, in_=rs_out[c * JB : (c + 1) * JB, :])
                Rf = tmp2.tile([128, DIM], dt.float32, tag="Rf")
                nc.vector.tensor_copy(out=Rf, in_=R)
                st = tmp2.tile([128, 6], dt.float32, tag="st")
                nc.vector.bn_stats(out=st, in_=Rf)
                mv = tmp2.tile([128, 2], dt.float32, tag="mv")
                nc.vector.bn_aggr(out=mv, in_=st)
                sd = tmp2.tile([128, 1], dt.float32, tag="sd")
                nc.scalar.activation(out=sd, in_=mv[:, 1:2], func=AF.Ln, bias=epsP)
                nc.scalar.activation(out=sd, in_=sd, func=AF.Exp, scale=-0.5)
                nc.vector.tensor_scalar(
                    out=Rf,
                    in0=Rf,
                    scalar1=mv[:, 0:1],
                    scalar2=sd,
                    op0=OP.subtract,
                    op1=OP.mult,
                )
                R2 = tmp2.tile([128, DIM], dt.float32, tag="R2")
                nc.vector.tensor_mul(out=R2, in0=Rf, in1=w_b)
                nc.vector.tensor_add(out=R2, in0=R2, in1=b_b)
                nc.sync.dma_start(out=out_ext[c * JB : (c + 1) * JB, :], in_=R2)

            state = {}

            def make_drain(pc, pavp):
                def cb():
                    state["OS"] = (pc, drain_avp(pc, pavp))
                return cb

            prev_avp = None
            for c in range(NIC):
                cb = make_drain(c - 1, prev_avp) if prev_avp is not None else None
                prev_avp = attention_block(c, cb)
                if "OS" in state:
                    tail_block(*state.pop("OS"))
                if c >= 2:
                    dw_chunk(c - 2)
            OS7 = drain_avp(NIC - 1, prev_avp)
            tail_block(NIC - 1, OS7)
            dw_chunk(NIC - 2)
            dw_chunk(NIC - 1)
            for c in range(NIC):
                stageD_chunk(c)

    return nc


_cached = {}


def _get_nc(flags):
    if flags not in _cached:
        nc = _build(flags)
        nc.finalize()
        _cached[flags] = nc
    return _cached[flags]


def _make_in_maps(inputs, flags):
    import ml_dtypes

    qk_w_fold, qk_general, o_general = flags
    x = np.asarray(inputs["x"], np.float32)
    pe_w = np.asarray(inputs["pe_w"], np.float32)
    pe_b = np.asarray(inputs["pe_b"], np.float32)
    qkv_dw = np.asarray(inputs["qkv_dw"], np.float32)
    qkv_pw = np.asarray(inputs["qkv_pw"], np.float32)
    out_dw = np.asarray(inputs["out_dw"], np.float32)
    out_pw = np.asarray(inputs["out_pw"], np.float32)
    nq_w, nq_b = np.asarray(inputs["nq_w"], np.float32), np.asarray(
        inputs["nq_b"], np.float32
    )
    nk_w, nk_b = np.asarray(inputs["nk_w"], np.float32), np.asarray(
        inputs["nk_b"], np.float32
    )
    no_w, no_b = np.asarray(inputs["no_w"], np.float32), np.asarray(
        inputs["no_b"], np.float32
    )
    ln_w, ln_b = np.asarray(inputs["ln_w"], np.float32), np.asarray(
        inputs["ln_b"], np.float32
    )

    gx = np.linspace(0.0, 1.0, Hs, dtype=np.float32)
    gy = np.linspace(0.0, 1.0, Ws, dtype=np.float32)
    meshb = np.stack(
        [np.repeat(gx, Ws), np.tile(gy, Hs), np.ones(N, np.float32)]
    ).astype(ml_dtypes.bfloat16)
    pewT = np.stack([pe_w[:, 0], pe_w[:, 1], pe_b]).astype(ml_dtypes.bfloat16)

    idx = np.arange(DH)
    in_maps = []
    for core in range(8):
        b, h = core // 4, core % 4
        rows = h + HEADS * idx
        qdiags = np.zeros((DIM, 9, DIM), np.float32)
        taps = qkv_dw.reshape(DIM, 9)
        for t in range(9):
            qdiags[np.arange(DIM), t, np.arange(DIM)] = taps[:, t]
        odiags = np.zeros((DH, 9, DH), np.float32)
        otaps = out_dw[rows].reshape(DH, 9).copy()
        if not o_general:
            otaps *= no_w[h][:, None]
        for t in range(9):
            odiags[idx, t, idx] = otaps[:, t]
        pwqT = np.ascontiguousarray(qkv_pw[rows, :].T)  # [DIM, DH]
        pwkT = np.ascontiguousarray(qkv_pw[DIM * 2 + rows, :].T)
        pwvT = np.ascontiguousarray(qkv_pw[DIM * 4 + rows, :].T)
        m = {
            "x": np.ascontiguousarray(x[b].reshape(DIM, N)),
            "meshb": meshb,
            "pewT": pewT,
            "qdiags": np.ascontiguousarray(qdiags.reshape(DIM, 9 * DIM)),
            "pwq2": np.ascontiguousarray(
                np.concatenate([pwqT, pwqT], 1) - pwqT.mean(1, keepdims=True)
            ),
            "pwk2": np.ascontiguousarray(
                np.concatenate([pwkT, pwkT], 1) - pwkT.mean(1, keepdims=True)
            ),
            "pwv": pwvT,
            "ones_var": np.full((DH, 128), 1.0 / DH, np.float32),
            "onesr1": np.ones((1, DH), np.float32),
            "odiags": np.ascontiguousarray(odiags.reshape(DH, 9 * DH)).astype(
                ml_dtypes.bfloat16
            ),
            "opw": np.ascontiguousarray(out_pw[:, rows].T).astype(ml_dtypes.bfloat16),
            "ln2w": np.ascontiguousarray(ln_w[None, :]),
            "ln2b": np.ascontiguousarray(ln_b[None, :]),
            "zpad": np.zeros((DIM, G), np.float32),
        }
        if qk_w_fold or qk_general:
            m["wqk"] = np.ascontiguousarray(
                np.tile(nq_w[h] * (1.0 if qk_general else nk_w[h]), 2)[:, None]
            )
            m["bq_t"] = np.ascontiguousarray(np.tile(nq_b[h], 2)[:, None])
            m["wk_t"] = np.ascontiguousarray(np.tile(nk_w[h], 2)[:, None])
            m["bk_t"] = np.ascontiguousarray(np.tile(nk_b[h], 2)[:, None])
        if o_general:
            m["wo_t"] = np.ascontiguousarray(no_w[h][:, None])
            m["bo_t"] = np.ascontiguousarray(no_b[h][:, None])
        in_maps.append(m)
    return in_maps


def _flags(inputs):
    nq_w = np.asarray(inputs["nq_w"], np.float32)
    nq_b = np.asarray(inputs["nq_b"], np.float32)
    nk_w = np.asarray(inputs["nk_w"], np.float32)
    nk_b = np.asarray(inputs["nk_b"], np.float32)
    no_w = np.asarray(inputs["no_w"], np.float32)
    no_b = np.asarray(inputs["no_b"], np.float32)
    qk_b_zero = np.all(nq_b == 0) and np.all(nk_b == 0)
    qk_w_one = np.all(nq_w == 1) and np.all(nk_w == 1)
    qk_general = not qk_b_zero
    qk_w_fold = qk_b_zero and not qk_w_one
    o_general = not np.all(no_b == 0)
    return (qk_w_fold, qk_general, o_general)


def run_on_device(inputs, **kw):
    flags = _flags(inputs)
    nc = _get_nc(flags)
    in_maps = _make_in_maps(inputs, flags)
    res = run_bass_kernel_spmd(nc, in_maps, core_ids=list(range(8)), **kw)
    out = np.zeros((B, DIM, N), np.float32)
    for core in range(8):
        b, h = core // 4, core % 4
        o = res.results[core]["out"]  # 4 RS pieces of 256 positions
        for p in range(4):
            g0 = p * 1024 + h * 256
            out[b][:, g0 : g0 + 256] = o[p * 256 : (p + 1) * 256].T
    return out.reshape(B, DIM, Hs, Ws), res


def kernel(**inputs):
    out, _ = run_on_device(inputs)
    return out
